# revision 11
# baseline (speedup 1.0000x reference)
"""MoE FFN (8 experts, top-2) on 8 TRN2 NeuronCores, expert-parallel.

Strategy:
  - Host: router (fp64 logits -> softmax -> top-2 -> renormalized combine
    weights), gather each expert's assigned tokens, pad to a common
    capacity C (SPMD: one program, per-core inputs).
  - Core e: full SwiGLU FFN for expert e over its C tokens in bf16
    (full PE rate, fp32 PSUM accumulate), combine-weight scaling fused
    into the PSUM evacuation; outputs [C, 1024] fp32.
  - Host: scatter-add per-expert outputs back into [B, S, D].

Device kernel structure (single pass over all weights per invocation):
  - x kept resident in SBUF for the whole kernel: [128, 8, C] bf16.
  - d_inner split into 4 groups of 8 h-tiles (128 rows each). Per group:
      mm1: for each h-tile, gate/up chains over 8 k-tiles (moving dim =
           tokens, 512 wide), SiLU (scalar) * up (vector) -> hbuf bf16.
      mm2: for each (token-subtile, dout-half) output tile, one PSUM
           chain of 8 matmuls over the group's h-tiles; evacuation does
           y_acc = psum * cw (+ y_acc), last group DMAs y_acc out.
  - Weights stream exactly once per invocation (~25 MB bf16/core), far
    under the ~700us of tensor work -> fully hidden.

Layouts (host-prepared, DMA-friendly, all weights/x in bf16):
  xT   [128, 8, C]          x[idx].T tiled [d_sub][k][t]
  guw  [32, 128, 2, 8, 128] gate/up ^T tiled: [h_tile][d_sub][g|u][k][h]
  dw   [4, 128, 8, 2, 512]  down^T tiled: [h_group][h_sub][h_tile][d_half][dout]
  cwT  [128, C/128]         combine weights fp32, partition-major
"""
import sys, os
for p in ("/opt/trn_rl_repo", os.path.join(os.path.dirname(os.path.abspath(__file__)))):
    if p not in sys.path:
        sys.path.insert(0, p)
import numpy as np

D_MODEL = 1024
D_INNER = 4096
N_EXPERTS = 8
TOP_K = 2
H_TILES = D_INNER // 128  # 32
K_TILES = D_MODEL // 128  # 8
HG = 8                    # h-tiles per group (PSUM chain depth in mm2)
N_HG = H_TILES // HG      # 4


def _build_nc(C: int, reps: int = 1):
    import concourse.mybir as mybir
    import concourse.tile as tile
    from concourse import bacc
    from contextlib import nullcontext

    f32 = mybir.dt.float32
    bf16 = mybir.dt.bfloat16
    Silu = mybir.ActivationFunctionType.Silu
    Mult = mybir.AluOpType.mult
    Add = mybir.AluOpType.add

    assert C % 32 == 0
    NT = (C + 127) // 128
    # moving-dim groups for the gate/up matmuls (tokens, <=512 per PSUM
    # bank). Equal-ish sizes: a tiny tail group would be LDWEIGHTS-bound.
    n_mg = (C + 511) // 512
    base = (C // n_mg) // 32 * 32
    sizes = [base] * n_mg
    rem = C - base * n_mg
    i = 0
    while rem > 0:
        sizes[i] += 32
        rem -= 32
        i = (i + 1) % n_mg
    mgroups = []
    t = 0
    for g in sizes:
        mgroups.append((t, g))
        t += g
    # split x into two SBUF tiles at a group boundary so compute can
    # start after the first half's DMA lands
    xsplit = mgroups[(n_mg + 1) // 2][0]

    nc = bacc.Bacc(None, target_bir_lowering=False)
    xT_d = nc.dram_tensor("xT", [128, K_TILES, C], bf16, kind="ExternalInput")
    guw_d = nc.dram_tensor("guw", [H_TILES, 128, 2, K_TILES, 128], bf16, kind="ExternalInput")
    dw_d = nc.dram_tensor("dw", [N_HG, 128, HG, 2, 512], bf16, kind="ExternalInput")
    cw_d = nc.dram_tensor("cwT", [128, NT], f32, kind="ExternalInput")
    y_d = nc.dram_tensor("y", [C, D_MODEL], f32, kind="ExternalOutput")

    with tile.TileContext(nc) as tc:
        with (
            tc.tile_pool(name="xt", bufs=1) as xt_pool,
            tc.tile_pool(name="wgt", bufs=3) as wgt_pool,
            tc.tile_pool(name="dwp", bufs=2) as dw_pool,
            tc.tile_pool(name="hb", bufs=1) as hb_pool,
            tc.tile_pool(name="sg", bufs=3) as sg_pool,
            tc.tile_pool(name="ya", bufs=1) as ya_pool,
            tc.tile_pool(name="cw", bufs=1) as cw_pool,
            tc.tile_pool(name="ps1", bufs=5, space="PSUM") as ps1,
            tc.tile_pool(name="ps2", bufs=3, space="PSUM") as ps2,
        ):
            cw_sb = cw_pool.tile([128, NT], f32)
            nc.sync.dma_start(cw_sb[:], cw_d[:])
            yacc = ya_pool.tile([128, NT, D_MODEL], f32)

            rep_ctx = tc.For_i(0, reps, 1) if reps > 1 else nullcontext()
            with rep_ctx:
                xta = xt_pool.tile([128, K_TILES, xsplit], bf16, tag="xta", name="xta")
                nc.sync.dma_start(xta[:], xT_d[:, :, 0:xsplit])
                xtb = xt_pool.tile([128, K_TILES, C - xsplit], bf16, tag="xtb", name="xtb")
                nc.sync.dma_start(xtb[:], xT_d[:, :, xsplit:C])

                def xslice(t0, gsz):
                    if t0 < xsplit:
                        assert t0 + gsz <= xsplit
                        return xta, slice(t0, t0 + gsz)
                    return xtb, slice(t0 - xsplit, t0 - xsplit + gsz)

                for hg in range(N_HG):
                    hbuf = hb_pool.tile([128, HG, C], bf16, tag="hb")
                    dwt = dw_pool.tile([128, HG, 2, 512], bf16, tag="dw")
                    nc.sync.dma_start(dwt[:], dw_d[hg])
                    # ---- mm1: gate/up + SwiGLU for this group's 8 h-tiles
                    for i in range(HG):
                        hi = hg * HG + i
                        guw = wgt_pool.tile([128, 2, K_TILES, 128], bf16, tag="w")
                        nc.sync.dma_start(guw[:], guw_d[hi])
                        # process moving groups in pairs so consecutive
                        # matmuls share the same stationary operand (one
                        # weight load serves both)
                        for p0 in range(0, len(mgroups), 2):
                            pair = mgroups[p0:p0 + 2]
                            xts = [xslice(t0, gsz) for (t0, gsz) in pair]
                            pgs, pus = [], []
                            for j, (t0, gsz) in enumerate(pair):
                                pgs.append(ps1.tile([128, gsz], f32, tag="p1",
                                                    name=f"pg{j}", padded_shape=[128, 512]))
                            for k in range(K_TILES):
                                for j in range(len(pair)):
                                    xtile, xs = xts[j]
                                    nc.tensor.matmul(pgs[j][:], guw[:, 0, k, :], xtile[:, k, xs],
                                                     start=(k == 0), stop=(k == K_TILES - 1))
                            for j, (t0, gsz) in enumerate(pair):
                                pus.append(ps1.tile([128, gsz], f32, tag="p1",
                                                    name=f"pu{j}", padded_shape=[128, 512]))
                            for k in range(K_TILES):
                                for j in range(len(pair)):
                                    xtile, xs = xts[j]
                                    nc.tensor.matmul(pus[j][:], guw[:, 1, k, :], xtile[:, k, xs],
                                                     start=(k == 0), stop=(k == K_TILES - 1))
                            for j, (t0, gsz) in enumerate(pair):
                                sg = sg_pool.tile([128, gsz], bf16, tag="sg", name="sg", padded_shape=[128, 512])
                                nc.scalar.activation(sg[:], pgs[j][:], Silu)
                                nc.vector.tensor_mul(hbuf[:, i, t0:t0 + gsz], sg[:], pus[j][:])
                    # ---- mm2: down-projection partial sums for this group
                    for ts in range(NT):
                        tw = min(128, C - ts * 128)
                        tsl = slice(ts * 128, ts * 128 + tw)
                        # both dout halves share the stationary token tile:
                        # interleave their accumulation chains
                        yps = [ps2.tile([128, 512], f32, tag="p2", name=f"yp{dh}")
                               for dh in range(2)]
                        for i in range(HG):
                            for dh in range(2):
                                nc.tensor.matmul(yps[dh][:tw, :], hbuf[:, i, tsl], dwt[:, i, dh, :],
                                                 start=(i == 0), stop=(i == HG - 1))
                        for dh in range(2):
                            ysl = yacc[:tw, ts, dh * 512:(dh + 1) * 512]
                            cws = cw_sb[:tw, ts:ts + 1]
                            if hg == 0:
                                nc.vector.tensor_scalar_mul(ysl, yps[dh][:tw, :], cws)
                            else:
                                nc.vector.scalar_tensor_tensor(ysl, yps[dh][:tw, :], cws, ysl, Mult, Add)
                            if hg == N_HG - 1:
                                nc.sync.dma_start(y_d[tsl, dh * 512:(dh + 1) * 512], ysl)
    nc.finalize()
    return nc


_NC_CACHE: dict = {}


def _get_nc(C: int):
    if C not in _NC_CACHE:
        _NC_CACHE[C] = _build_nc(C)
    return _NC_CACHE[C]


def _route(x2d: np.ndarray, router_w: np.ndarray, router_b: np.ndarray):
    """fp64 router: returns (idx_per_expert, cw_per_expert) lists."""
    logits = x2d.astype(np.float64) @ router_w.astype(np.float64).T + router_b.astype(np.float64)
    m = logits.max(axis=-1, keepdims=True)
    p = np.exp(logits - m)
    p /= p.sum(axis=-1, keepdims=True)
    # top-2 (jax.lax.top_k picks largest; softmax is monotonic in logits)
    i1 = np.argmax(p, axis=-1)
    p_masked = p.copy()
    p_masked[np.arange(p.shape[0]), i1] = -1.0
    i2 = np.argmax(p_masked, axis=-1)
    p1 = p[np.arange(p.shape[0]), i1]
    p2 = p[np.arange(p.shape[0]), i2]
    denom = p1 + p2
    w1 = p1 / denom
    w2 = p2 / denom
    idxs, cws = [], []
    for e in range(N_EXPERTS):
        sel1 = np.nonzero(i1 == e)[0]
        sel2 = np.nonzero(i2 == e)[0]
        idx = np.concatenate([sel1, sel2])
        cw = np.concatenate([w1[sel1], w2[sel2]])
        idxs.append(idx)
        cws.append(cw.astype(np.float32))
    return idxs, cws


def _prep_core_inputs(x2d, idxs, cws, gate_w, up_w, down_w, C):
    import ml_dtypes
    bf16 = ml_dtypes.bfloat16
    in_maps = []
    for e in range(N_EXPERTS):
        idx = idxs[e]
        n = len(idx)
        xe = np.zeros((C, D_MODEL), np.float32)
        xe[:n] = x2d[idx]
        xT = np.ascontiguousarray(
            xe.T.reshape(K_TILES, 128, C).transpose(1, 0, 2)).astype(bf16)
        g_t = gate_w[e].T.reshape(K_TILES, 128, H_TILES, 128).transpose(2, 1, 0, 3)
        u_t = up_w[e].T.reshape(K_TILES, 128, H_TILES, 128).transpose(2, 1, 0, 3)
        guw = np.ascontiguousarray(np.stack([g_t, u_t], axis=2)).astype(bf16)
        dw = np.ascontiguousarray(
            down_w[e].T.reshape(N_HG, HG, 128, 2, 512).transpose(0, 2, 1, 3, 4)).astype(bf16)
        NT = (C + 127) // 128
        cw = np.zeros((NT * 128,), np.float32)
        cw[:n] = cws[e]
        cwT = np.ascontiguousarray(cw.reshape(-1, 128).T)
        in_maps.append({"xT": xT, "guw": guw, "dw": dw, "cwT": cwT})
    return in_maps


def kernel(x, router_w, router_b, gate_w, up_w, down_w):
    from concourse.bass_utils import run_bass_kernel_spmd

    x = np.asarray(x, dtype=np.float32)
    router_w = np.asarray(router_w, dtype=np.float32)
    router_b = np.asarray(router_b, dtype=np.float32)
    gate_w = np.asarray(gate_w, dtype=np.float32)
    up_w = np.asarray(up_w, dtype=np.float32)
    down_w = np.asarray(down_w, dtype=np.float32)

    B, S, D = x.shape
    x2d = x.reshape(B * S, D)
    idxs, cws = _route(x2d, router_w, router_b)
    max_n = max(len(i) for i in idxs)
    C = max(256, ((max_n + 31) // 32) * 32)

    nc = _get_nc(C)
    in_maps = _prep_core_inputs(x2d, idxs, cws, gate_w, up_w, down_w, C)
    res = run_bass_kernel_spmd(nc, in_maps, core_ids=list(range(N_EXPERTS)), trace=False)

    out = np.zeros((B * S, D_MODEL), np.float32)
    for e in range(N_EXPERTS):
        n = len(idxs[e])
        np.add.at(out, idxs[e], res.results[e]["y"][:n])
    return out.reshape(B, S, D_MODEL)


# revision 13
# speedup vs baseline: 1.2674x; 1.2674x over previous
"""MoE FFN (8 experts, top-2) on 8 TRN2 NeuronCores, expert-parallel.

Strategy:
  - Host: router (fp64 logits -> softmax -> top-2 -> renormalized combine
    weights), gather each expert's assigned tokens, pad to a common
    capacity C (SPMD: one program, per-core inputs).
  - Core e: full SwiGLU FFN for expert e over its C tokens in bf16
    (full PE rate, fp32 PSUM accumulate), combine-weight scaling fused
    into the PSUM evacuation; outputs [C, 1024] fp32.
  - Host: scatter-add per-expert outputs back into [B, S, D].

Device kernel structure (single pass over all weights per invocation):
  - x kept resident in SBUF for the whole kernel: [128, 8, C] bf16.
  - d_inner split into 4 groups of 8 h-tiles (128 rows each). Per group:
      mm1: for each h-tile, gate/up chains over 8 k-tiles (moving dim =
           tokens, 512 wide), SiLU (scalar) * up (vector) -> hbuf bf16.
      mm2: for each (token-subtile, dout-half) output tile, one PSUM
           chain of 8 matmuls over the group's h-tiles; evacuation does
           y_acc = psum * cw (+ y_acc), last group DMAs y_acc out.
  - Weights stream exactly once per invocation (~25 MB bf16/core), far
    under the ~700us of tensor work -> fully hidden.

Layouts (host-prepared, DMA-friendly, all weights/x in bf16):
  xT   [128, 8, C]          x[idx].T tiled [d_sub][k][t]
  guw  [32, 128, 2, 8, 128] gate/up ^T tiled: [h_tile][d_sub][g|u][k][h]
  dw   [4, 128, 8, 2, 512]  down^T tiled: [h_group][h_sub][h_tile][d_half][dout]
  cwT  [128, C/128]         combine weights fp32, partition-major
"""
import sys, os
for p in ("/opt/trn_rl_repo", os.path.join(os.path.dirname(os.path.abspath(__file__)))):
    if p not in sys.path:
        sys.path.insert(0, p)
import numpy as np

D_MODEL = 1024
D_INNER = 4096
N_EXPERTS = 8
TOP_K = 2
H_TILES = D_INNER // 128  # 32
K_TILES = D_MODEL // 128  # 8
HG = 8                    # h-tiles per group (PSUM chain depth in mm2)
N_HG = H_TILES // HG      # 4


def _build_nc(C: int, reps: int = 1):
    import concourse.mybir as mybir
    import concourse.tile as tile
    from concourse import bacc
    from contextlib import nullcontext

    f32 = mybir.dt.float32
    bf16 = mybir.dt.bfloat16
    Silu = mybir.ActivationFunctionType.Silu
    Mult = mybir.AluOpType.mult
    Add = mybir.AluOpType.add

    assert C % 32 == 0
    NT = (C + 127) // 128
    # moving-dim groups for the gate/up matmuls (tokens, <=512 per PSUM
    # bank). Equal-ish sizes: a tiny tail group would be LDWEIGHTS-bound.
    n_mg = (C + 511) // 512
    base = (C // n_mg) // 32 * 32
    sizes = [base] * n_mg
    rem = C - base * n_mg
    i = 0
    while rem > 0:
        sizes[i] += 32
        rem -= 32
        i = (i + 1) % n_mg
    mgroups = []
    t = 0
    for g in sizes:
        mgroups.append((t, g))
        t += g
    # split x into two SBUF tiles at a group boundary so compute can
    # start after the first half's DMA lands
    xsplit = mgroups[(n_mg + 1) // 2][0]

    nc = bacc.Bacc(None, target_bir_lowering=False)
    xT_d = nc.dram_tensor("xT", [128, K_TILES, C], bf16, kind="ExternalInput")
    guw_d = nc.dram_tensor("guw", [H_TILES, 128, 2, K_TILES, 128], bf16, kind="ExternalInput")
    dw_d = nc.dram_tensor("dw", [N_HG, 128, HG, 2, 512], bf16, kind="ExternalInput")
    cw_d = nc.dram_tensor("cwT", [128, NT], f32, kind="ExternalInput")
    y_d = nc.dram_tensor("y", [C, D_MODEL], f32, kind="ExternalOutput")

    with tile.TileContext(nc) as tc:
        with (
            tc.tile_pool(name="xt", bufs=1) as xt_pool,
            tc.tile_pool(name="wgt", bufs=3) as wgt_pool,
            tc.tile_pool(name="dwp", bufs=2) as dw_pool,
            tc.tile_pool(name="hb", bufs=1) as hb_pool,
            tc.tile_pool(name="sg", bufs=3) as sg_pool,
            tc.tile_pool(name="ya", bufs=1) as ya_pool,
            tc.tile_pool(name="cw", bufs=1) as cw_pool,
            tc.tile_pool(name="ps1", bufs=5, space="PSUM") as ps1,
            tc.tile_pool(name="ps2", bufs=3, space="PSUM") as ps2,
        ):
            cw_sb = cw_pool.tile([128, NT], f32)
            nc.sync.dma_start(cw_sb[:], cw_d[:])
            yacc = ya_pool.tile([128, NT, D_MODEL], f32)

            rep_ctx = tc.For_i(0, reps, 1) if reps > 1 else nullcontext()
            with rep_ctx:
                xta = xt_pool.tile([128, K_TILES, xsplit], bf16, tag="xta", name="xta")
                nc.sync.dma_start(xta[:], xT_d[:, :, 0:xsplit])
                xtb = xt_pool.tile([128, K_TILES, C - xsplit], bf16, tag="xtb", name="xtb")
                nc.sync.dma_start(xtb[:], xT_d[:, :, xsplit:C])

                def xslice(t0, gsz):
                    if t0 < xsplit:
                        assert t0 + gsz <= xsplit
                        return xta, slice(t0, t0 + gsz)
                    return xtb, slice(t0 - xsplit, t0 - xsplit + gsz)

                for hg in range(N_HG):
                    hbuf = hb_pool.tile([128, HG, C], bf16, tag="hb")
                    dwt = dw_pool.tile([128, HG, 2, 512], bf16, tag="dw")
                    nc.sync.dma_start(dwt[:], dw_d[hg])
                    # ---- mm1: gate/up + SwiGLU for this group's 8 h-tiles
                    for i in range(HG):
                        hi = hg * HG + i
                        guw = wgt_pool.tile([128, 2, K_TILES, 128], bf16, tag="w")
                        # scalar-engine HWDGE ring: keeps the weight stream
                        # (and y below) off the sync ring that feeds x/dw,
                        # so neither blocks the other head-of-line
                        nc.scalar.dma_start(guw[:], guw_d[hi])
                        # process moving groups in pairs so consecutive
                        # matmuls share the same stationary operand (one
                        # weight load serves both)
                        for p0 in range(0, len(mgroups), 2):
                            pair = mgroups[p0:p0 + 2]
                            xts = [xslice(t0, gsz) for (t0, gsz) in pair]
                            pgs, pus = [], []
                            for j, (t0, gsz) in enumerate(pair):
                                pgs.append(ps1.tile([128, gsz], f32, tag="p1",
                                                    name=f"pg{j}", padded_shape=[128, 512]))
                            for k in range(K_TILES):
                                for j in range(len(pair)):
                                    xtile, xs = xts[j]
                                    nc.tensor.matmul(pgs[j][:], guw[:, 0, k, :], xtile[:, k, xs],
                                                     start=(k == 0), stop=(k == K_TILES - 1))
                            for j, (t0, gsz) in enumerate(pair):
                                pus.append(ps1.tile([128, gsz], f32, tag="p1",
                                                    name=f"pu{j}", padded_shape=[128, 512]))
                            for k in range(K_TILES):
                                for j in range(len(pair)):
                                    xtile, xs = xts[j]
                                    nc.tensor.matmul(pus[j][:], guw[:, 1, k, :], xtile[:, k, xs],
                                                     start=(k == 0), stop=(k == K_TILES - 1))
                            for j, (t0, gsz) in enumerate(pair):
                                sg = sg_pool.tile([128, gsz], bf16, tag="sg", name="sg", padded_shape=[128, 512])
                                nc.scalar.activation(sg[:], pgs[j][:], Silu)
                                nc.vector.tensor_mul(hbuf[:, i, t0:t0 + gsz], sg[:], pus[j][:])
                    # ---- mm2: down-projection partial sums for this group
                    for ts in range(NT):
                        tw = min(128, C - ts * 128)
                        tsl = slice(ts * 128, ts * 128 + tw)
                        # both dout halves share the stationary token tile:
                        # interleave their accumulation chains
                        yps = [ps2.tile([128, 512], f32, tag="p2", name=f"yp{dh}")
                               for dh in range(2)]
                        for i in range(HG):
                            for dh in range(2):
                                nc.tensor.matmul(yps[dh][:tw, :], hbuf[:, i, tsl], dwt[:, i, dh, :],
                                                 start=(i == 0), stop=(i == HG - 1))
                        for dh in range(2):
                            ysl = yacc[:tw, ts, dh * 512:(dh + 1) * 512]
                            cws = cw_sb[:tw, ts:ts + 1]
                            if hg == 0:
                                nc.vector.tensor_scalar_mul(ysl, yps[dh][:tw, :], cws)
                            else:
                                nc.vector.scalar_tensor_tensor(ysl, yps[dh][:tw, :], cws, ysl, Mult, Add)
                            if hg == N_HG - 1:
                                nc.scalar.dma_start(y_d[tsl, dh * 512:(dh + 1) * 512], ysl)
    nc.finalize()
    return nc


_NC_CACHE: dict = {}


def _get_nc(C: int):
    if C not in _NC_CACHE:
        _NC_CACHE[C] = _build_nc(C)
    return _NC_CACHE[C]


def _route(x2d: np.ndarray, router_w: np.ndarray, router_b: np.ndarray):
    """fp64 router: returns (idx_per_expert, cw_per_expert) lists."""
    logits = x2d.astype(np.float64) @ router_w.astype(np.float64).T + router_b.astype(np.float64)
    m = logits.max(axis=-1, keepdims=True)
    p = np.exp(logits - m)
    p /= p.sum(axis=-1, keepdims=True)
    # top-2 (jax.lax.top_k picks largest; softmax is monotonic in logits)
    i1 = np.argmax(p, axis=-1)
    p_masked = p.copy()
    p_masked[np.arange(p.shape[0]), i1] = -1.0
    i2 = np.argmax(p_masked, axis=-1)
    p1 = p[np.arange(p.shape[0]), i1]
    p2 = p[np.arange(p.shape[0]), i2]
    denom = p1 + p2
    w1 = p1 / denom
    w2 = p2 / denom
    idxs, cws = [], []
    for e in range(N_EXPERTS):
        sel1 = np.nonzero(i1 == e)[0]
        sel2 = np.nonzero(i2 == e)[0]
        idx = np.concatenate([sel1, sel2])
        cw = np.concatenate([w1[sel1], w2[sel2]])
        idxs.append(idx)
        cws.append(cw.astype(np.float32))
    return idxs, cws


def _prep_core_inputs(x2d, idxs, cws, gate_w, up_w, down_w, C):
    import ml_dtypes
    bf16 = ml_dtypes.bfloat16
    in_maps = []
    for e in range(N_EXPERTS):
        idx = idxs[e]
        n = len(idx)
        xe = np.zeros((C, D_MODEL), np.float32)
        xe[:n] = x2d[idx]
        xT = np.ascontiguousarray(
            xe.T.reshape(K_TILES, 128, C).transpose(1, 0, 2)).astype(bf16)
        g_t = gate_w[e].T.reshape(K_TILES, 128, H_TILES, 128).transpose(2, 1, 0, 3)
        u_t = up_w[e].T.reshape(K_TILES, 128, H_TILES, 128).transpose(2, 1, 0, 3)
        guw = np.ascontiguousarray(np.stack([g_t, u_t], axis=2)).astype(bf16)
        dw = np.ascontiguousarray(
            down_w[e].T.reshape(N_HG, HG, 128, 2, 512).transpose(0, 2, 1, 3, 4)).astype(bf16)
        NT = (C + 127) // 128
        cw = np.zeros((NT * 128,), np.float32)
        cw[:n] = cws[e]
        cwT = np.ascontiguousarray(cw.reshape(-1, 128).T)
        in_maps.append({"xT": xT, "guw": guw, "dw": dw, "cwT": cwT})
    return in_maps


def kernel(x, router_w, router_b, gate_w, up_w, down_w):
    from concourse.bass_utils import run_bass_kernel_spmd

    x = np.asarray(x, dtype=np.float32)
    router_w = np.asarray(router_w, dtype=np.float32)
    router_b = np.asarray(router_b, dtype=np.float32)
    gate_w = np.asarray(gate_w, dtype=np.float32)
    up_w = np.asarray(up_w, dtype=np.float32)
    down_w = np.asarray(down_w, dtype=np.float32)

    B, S, D = x.shape
    x2d = x.reshape(B * S, D)
    idxs, cws = _route(x2d, router_w, router_b)
    max_n = max(len(i) for i in idxs)
    C = max(256, ((max_n + 31) // 32) * 32)

    nc = _get_nc(C)
    in_maps = _prep_core_inputs(x2d, idxs, cws, gate_w, up_w, down_w, C)
    res = run_bass_kernel_spmd(nc, in_maps, core_ids=list(range(N_EXPERTS)), trace=False)

    out = np.zeros((B * S, D_MODEL), np.float32)
    for e in range(N_EXPERTS):
        n = len(idxs[e])
        np.add.at(out, idxs[e], res.results[e]["y"][:n])
    return out.reshape(B, S, D_MODEL)
